# revision 1
# baseline (speedup 1.0000x reference)
"""CQAttention (QANet context-query attention) Trainium2 kernel.

Problem: B=64, H=256, Lc=2048, Lq=256.
  S[b,i,j] = (Ct@w1)[i] + (Qt@w2)[j] + sum_h Ct[i,h]*w3[h]*Qt[j,h]
  S_row = softmax_j(masked), S_col = softmax_i(masked)
  A = S_row @ Qt ; Bt = S_row @ (S_col^T @ Ct)
  out[b] = [Ct; A; Ct*A; Ct*Bt]^T  -> [B, 4H, Lc]

Strategy: data-parallel over batch (8 per core x 8 cores). Per batch:
  - host precomputes r=Ct@w1, c=Qt@w2, mask-folded bias columns, Qt, Q*w3,
    and bf16 Ct augmented with a ones column (for column-softmax sums).
  - S^T [j,i] on PE (lhsT=Q*w3, rhs=C) -> ACT exp with per-partition bias
    (c[j] - 1e30*qmask[j]) -> Pr^T (float32r, unnormalized).
  - row sums replicated across partitions via ones-matmul; reciprocal on DVE.
  - S [i,j] on PE (lhsT=C, rhs=Q*w3) -> ACT exp with bias
    (r[i] - 1e30*cmask[i]) -> Pc (bf16).
  - X_aug = Pc^T @ [Ct|1] (bf16) gives col-attention numerator + colsum;
    normalized on eviction (tensor_scalar by 1/colsum).
  - A^T = Qt^T @ Pr^T and Bt^T = X^T @ Pr^T (f32r), row-normalized by the
    replicated reciprocal during PSUM eviction (DVE tensor_tensor).
  - epilogue products with C split across GPSIMD/DVE; 1MB output DMAs.
"""

import numpy as np

B, H, LC, LQ = 64, 256, 2048, 256
NCORES = 8
NB = B // NCORES  # batches per core
NEG = 1.0e30

HC = H // 128   # 2 h-chunks
JC = LQ // 128  # 2 j-chunks
IC = LC // 128  # 16 i-chunks
IT = LC // 512  # 4 i-tiles
HA = H + 1      # augmented (ones column) width

_CACHE = {}


def _build():
    import concourse.bacc as bacc
    import concourse.mybir as mybir
    import concourse.tile as tile
    from contextlib import ExitStack

    F32 = mybir.dt.float32
    F32R = mybir.dt.float32r
    F16 = mybir.dt.float16
    BF16 = mybir.dt.bfloat16
    AF = mybir.ActivationFunctionType
    MUL = mybir.AluOpType.mult

    nc = bacc.Bacc("TRN2", target_bir_lowering=False, debug=False,
                   enable_asserts=False)

    c32 = nc.dram_tensor("c32", [NB, H, LC], F16, kind="ExternalInput").ap()
    q3 = nc.dram_tensor("q3", [NB, H, LQ], F16, kind="ExternalInput").ap()
    qt = nc.dram_tensor("qt", [NB, LQ, H], F32R, kind="ExternalInput").ap()
    rcb = nc.dram_tensor("rcb", [NB, 128, IC + JC], F32, kind="ExternalInput").ap()
    kid = nc.dram_tensor("kid", [128, 128], F16, kind="ExternalInput").ap()
    out = nc.dram_tensor("out", [NB, 4 * H, LC], F32, kind="ExternalOutput").ap()

    with tile.TileContext(nc) as tc:
        with ExitStack() as ctx:
            konst = ctx.enter_context(tc.tile_pool(name="konst", bufs=1))
            crpool = ctx.enter_context(tc.tile_pool(name="crpool", bufs=3))
            ctpool = ctx.enter_context(tc.tile_pool(name="ctpool", bufs=2))
            qpool = ctx.enter_context(tc.tile_pool(name="qpool", bufs=3))
            prpool = ctx.enter_context(tc.tile_pool(name="prpool", bufs=2))
            pcpool = ctx.enter_context(tc.tile_pool(name="pcpool", bufs=2))
            rrpool = ctx.enter_context(tc.tile_pool(name="rrpool", bufs=2))
            xpool = ctx.enter_context(tc.tile_pool(name="xpool", bufs=2))
            opool = ctx.enter_context(tc.tile_pool(name="opool", bufs=10))
            small = ctx.enter_context(tc.tile_pool(name="small", bufs=6))
            mm_ps = ctx.enter_context(tc.tile_pool(name="mm_ps", bufs=5, space="PSUM"))
            s3_ps = ctx.enter_context(tc.tile_pool(name="s3_ps", bufs=2, space="PSUM"))
            x_ps = ctx.enter_context(tc.tile_pool(name="x_ps", bufs=1, space="PSUM"))

            ones32 = konst.tile([128, 128], F32)
            nc.vector.memset(ones32[:], 1.0)
            ones_r = konst.tile([128, 128], F32R)
            nc.vector.tensor_copy(ones_r[:], ones32[:])
            kid_sb = konst.tile([128, 128], F16)
            nc.sync.dma_start(kid_sb[:], kid[:])

            def load_batch(b):
                q3sb = qpool.tile([128, HC * LQ], F16, tag="q3sb")
                nc.sync.dma_start(
                    q3sb[:].rearrange("p (c j) -> p c j", c=HC),
                    q3[b].rearrange("(c p) j -> p c j", p=128))
                crsb = crpool.tile([128, HC * LC], F16, tag="crsb")
                for kc in range(HC):
                    nc.sync.dma_start(
                        crsb[:, kc * LC:(kc + 1) * LC],
                        c32[b, kc * 128:(kc + 1) * 128, :])
                qtsb = qpool.tile([128, JC * H], F32R, tag="qtsb")
                nc.sync.dma_start(
                    qtsb[:].rearrange("p (c h) -> p c h", c=JC),
                    qt[b].rearrange("(c p) h -> p c h", p=128))
                rcbsb = small.tile([128, IC + JC], F32, tag="rcbsb")
                nc.sync.dma_start(rcbsb[:], rcb[b])
                return crsb, q3sb, qtsb, rcbsb[:, 0:IC], rcbsb[:, IC:IC + JC]

            tiles = load_batch(0)
            for b in range(NB):
                crsb, q3sb, qtsb, rmsb, cbsb = tiles
                cf = crsb[:]  # fp16 C for the epilogue products
                if b + 1 < NB:
                    tiles = load_batch(b + 1)

                # O1: C section, cast-store fp16 -> f32 via SWDGE (early)
                for hc in range(HC):
                    nc.gpsimd.dma_start(out[b, hc * 128:(hc + 1) * 128, :],
                                        cf[:, hc * LC:(hc + 1) * LC])

                # ---- row path: S^T tiles -> exp -> Pr^T; replicated rowsums ----
                prt = prpool.tile([128, JC * LC], F32R, tag="prt")
                rrep = rrpool.tile([128, LC], F32, tag="rrep")
                for it in range(IT):
                    for jc in range(JC):
                        ps = mm_ps.tile([128, 512], F32, tag="mm")
                        for kc in range(HC):
                            nc.tensor.matmul(
                                ps[:],
                                q3sb[:, kc * LQ + jc * 128:kc * LQ + (jc + 1) * 128],
                                crsb[:, kc * LC + it * 512:kc * LC + (it + 1) * 512],
                                start=(kc == 0), stop=(kc == HC - 1))
                        nc.scalar.activation(
                            prt[:, jc * LC + it * 512:jc * LC + (it + 1) * 512],
                            ps[:], AF.Exp, bias=cbsb[:, jc:jc + 1])
                    rs = mm_ps.tile([128, 512], F32, tag="mm")
                    for jc in range(JC):
                        nc.tensor.matmul(
                            rs[:], ones_r[:],
                            prt[:, jc * LC + it * 512:jc * LC + (it + 1) * 512],
                            start=(jc == 0), stop=(jc == JC - 1))
                    nc.vector.reciprocal_approx_fast(rrep[:, it * 512:(it + 1) * 512], rs[:])

                # ---- col path: S tiles -> exp -> Pc (bf16) ----
                pc = pcpool.tile([128, IC * LQ], BF16, tag="pc")
                for ic in range(IC):
                    ps3 = s3_ps.tile([128, LQ], F32, tag="s3")
                    for kc in range(HC):
                        nc.tensor.matmul(
                            ps3[:],
                            crsb[:, kc * LC + ic * 128:kc * LC + (ic + 1) * 128],
                            q3sb[:, kc * LQ:(kc + 1) * LQ],
                            start=(kc == 0), stop=(kc == HC - 1))
                    nc.scalar.activation(
                        pc[:, ic * LQ:(ic + 1) * LQ],
                        ps3[:], AF.Exp, bias=rmsb[:, ic:ic + 1])

                # ---- assemble Ct_aug on-chip: PE transposes of C -> bf16 ----
                ctsb = ctpool.tile([128, IC * HA], BF16, tag="ctsb")
                ct3 = ctsb[:].rearrange("p (n h) -> p n h", n=IC)
                for g in range(IC // 2):
                    tp = mm_ps.tile([128, 512], F16, tag="mm")
                    for u in range(2):
                        ic = 2 * g + u
                        for kc in range(HC):
                            nc.tensor.transpose(
                                tp[:, (2 * u + kc) * 128:(2 * u + kc + 1) * 128],
                                crsb[:, kc * LC + ic * 128:kc * LC + (ic + 1) * 128],
                                kid_sb[:])
                    nc.scalar.copy(ct3[:, 2 * g:2 * g + 2, 0:H], tp[:])
                nc.vector.memset(ct3[:, :, H:HA], 1.0)

                # ---- M3: X_aug = Pc^T @ [Ct|1]; normalize by colsum ----
                xsb = xpool.tile([128, JC * H], F32R, tag="xsb")
                for jc in range(JC):
                    xps = x_ps.tile([128, HA], F32, tag="x")
                    for ic in range(IC):
                        nc.tensor.matmul(
                            xps[:],
                            pc[:, ic * LQ + jc * 128:ic * LQ + (jc + 1) * 128],
                            ctsb[:, ic * HA:(ic + 1) * HA],
                            start=(ic == 0), stop=(ic == IC - 1))
                    colr = small.tile([128, 1], F32, tag="colr")
                    nc.vector.reciprocal_approx_fast(colr[:], xps[:, H:H + 1])
                    nc.vector.tensor_scalar_mul(
                        xsb[:, jc * H:(jc + 1) * H], xps[:, 0:H], colr[:])

                # ---- M2/M4 + epilogue ----
                for hc in range(HC):
                    for it in range(IT):
                        i0, i1 = it * 512, (it + 1) * 512
                        o2 = opool.tile([128, 512], F32, tag="obuf")
                        o3 = opool.tile([128, 512], F32, tag="obuf")
                        o4 = opool.tile([128, 512], F32, tag="obuf")
                        aps = mm_ps.tile([128, 512], F32, tag="mm")
                        for jc in range(JC):
                            nc.tensor.matmul(
                                aps[:],
                                qtsb[:, jc * H + hc * 128:jc * H + (hc + 1) * 128],
                                prt[:, jc * LC + i0:jc * LC + i1],
                                start=(jc == 0), stop=(jc == JC - 1))
                        bps = mm_ps.tile([128, 512], F32, tag="mm")
                        for jc in range(JC):
                            nc.tensor.matmul(
                                bps[:],
                                xsb[:, jc * H + hc * 128:jc * H + (hc + 1) * 128],
                                prt[:, jc * LC + i0:jc * LC + i1],
                                start=(jc == 0), stop=(jc == JC - 1))
                        # O2 = A^T*rrep ; O4 = Bt^T*(C*rrep) ; O3 = O2*C
                        nc.vector.tensor_tensor(
                            o2[:], aps[:], rrep[:, i0:i1], MUL)
                        cr = small.tile([128, 512], F32, tag="cr")
                        nc.gpsimd.tensor_tensor(
                            cr[:], cf[:, hc * LC + i0:hc * LC + i1],
                            rrep[:, i0:i1], MUL)
                        nc.vector.tensor_tensor(o4[:], bps[:], cr[:], MUL)
                        nc.gpsimd.tensor_tensor(
                            o3[:], o2[:],
                            cf[:, hc * LC + i0:hc * LC + i1], MUL)
                        r0 = hc * 128
                        nc.sync.dma_start(out[b, H + r0:H + r0 + 128, i0:i1], o2[:])
                        nc.sync.dma_start(out[b, 2 * H + r0:2 * H + r0 + 128, i0:i1], o3[:])
                        nc.sync.dma_start(out[b, 3 * H + r0:3 * H + r0 + 128, i0:i1], o4[:])

    nc.compile()
    return nc


def _prep(C, Q, cmask, qmask, line_project):
    w1, w2, w3 = np.split(line_project.astype(np.float64), 3)
    r = np.einsum('bhi,h->bi', C.astype(np.float64), w1).astype(np.float32)
    c_ = np.einsum('bhj,h->bj', Q.astype(np.float64), w2).astype(np.float32)
    rm = (r - NEG * cmask).reshape(B, IC, 128).transpose(0, 2, 1)
    cb = (c_ - NEG * qmask).reshape(B, JC, 128).transpose(0, 2, 1)
    rcb = np.concatenate([rm, cb], axis=2).astype(np.float32)
    q3 = (Q * w3.astype(np.float32)[None, :, None]).astype(np.float16)
    qt = np.ascontiguousarray(Q.transpose(0, 2, 1))
    return rcb, q3, qt


def make_in_maps(C, Q, cmask, qmask, line_project):
    C = np.asarray(C, dtype=np.float32)
    Q = np.asarray(Q, dtype=np.float32)
    cmask = np.asarray(cmask, dtype=np.float32)
    qmask = np.asarray(qmask, dtype=np.float32)
    line_project = np.asarray(line_project, dtype=np.float32)
    rcb, q3, qt = _prep(C, Q, cmask, qmask, line_project)
    C16 = C.astype(np.float16)
    in_maps = []
    for core in range(NCORES):
        s = slice(core * NB, (core + 1) * NB)
        in_maps.append({
            "c32": np.ascontiguousarray(C16[s]),
            "q3": np.ascontiguousarray(q3[s]),
            "qt": np.ascontiguousarray(qt[s]),
            "rcb": np.ascontiguousarray(rcb[s]),
            "kid": np.eye(128, dtype=np.float16),
        })
    return in_maps


def kernel(C, Q, cmask, qmask, line_project):
    from concourse.bass_utils import run_bass_kernel_spmd

    in_maps = make_in_maps(C, Q, cmask, qmask, line_project)
    if "nc" not in _CACHE:
        _CACHE["nc"] = _build()
    nc = _CACHE["nc"]
    res = run_bass_kernel_spmd(nc, in_maps, core_ids=list(range(NCORES)))
    return np.concatenate([res.results[c]["out"] for c in range(NCORES)], axis=0)



# revision 6
# speedup vs baseline: 1.4314x; 1.4314x over previous
"""CQAttention (QANet context-query attention) Trainium2 kernel, v2.

Problem: B=64, H=256, Lc=2048, Lq=256.
  S[b,i,j] = r[i] + c[j] + S3[i,j],  S3 = sum_h Ct[i,h]*w3[h]*Qt[j,h]
  S_row = softmax_j(masked by qmask), S_col = softmax_i(masked by cmask)
  A = S_row @ Qt ; Bt = S_row @ (S_col^T @ Ct)
  out[b] = [Ct; A; Ct*A; Ct*Bt]^T  -> [B, 4H, Lc]

Strategy (data-parallel, 8 batches/core):
  - Rank-1 terms and masks are folded into host-prescaled operands:
      P0 = exp(S3 - 2)          (single exp; fp16)
      row path: A ~ (Qt*e^c*vq) vs P0^T; rowsum ~ (e^c*vq) vs P0^T
                (e^{r_i} and the global e^-2 cancel in the row softmax)
      col path: X = S_col^T@Ct via Ct*e^r*vc (+ e^r*vc ones-col for colsums)
  - Device: S3 on PE (fp16), exp on ACT, P0^T via PE transposes,
    X_aug / A^T / Bt^T / rowsum matmuls on PE, normalize on DVE/GPSIMD
    eviction, outputs shipped as fp16 A^T / Bt^T.
  - Host: assembles [C; A; C*A; C*Bt] (C section is the input verbatim)
    and upcasts to f32.
"""

import numpy as np

B, H, LC, LQ = 64, 256, 2048, 256
NCORES = 8
NB = B // NCORES

KC = 2    # h chunks of 128
JC = 2    # j chunks of 128
IC = 16   # i chunks of 128
IT = 4    # i tiles of 512
HA = H + 1
LAM = 2.0
S_EC = 32.0
S_ER = 4.0

_CACHE = {}


def _build():
    import concourse.bacc as bacc
    import concourse.mybir as mybir
    import concourse.tile as tile
    from contextlib import ExitStack

    F32 = mybir.dt.float32
    F16 = mybir.dt.float16
    AF = mybir.ActivationFunctionType
    MUL = mybir.AluOpType.mult

    nc = bacc.Bacc("TRN2", target_bir_lowering=False, debug=False,
                   enable_asserts=False)

    c16 = nc.dram_tensor("c16", [NB, 128, KC, LC], F16, kind="ExternalInput").ap()
    q316 = nc.dram_tensor("q316", [NB, 128, KC, LQ], F16, kind="ExternalInput").ap()
    qt16 = nc.dram_tensor("qt16", [NB, 128, JC, H], F16, kind="ExternalInput").ap()
    vqec = nc.dram_tensor("vqec", [NB, 128, JC, 128], F16, kind="ExternalInput").ap()
    ct16 = nc.dram_tensor("ct16", [NB, 128, IC, HA], F16, kind="ExternalInput").ap()
    kid = nc.dram_tensor("kid", [128, 128], F16, kind="ExternalInput").ap()
    a16 = nc.dram_tensor("a16", [NB, 128, KC, LC], F16, kind="ExternalOutput").ap()
    b16 = nc.dram_tensor("b16", [NB, 128, KC, LC], F16, kind="ExternalOutput").ap()

    with tile.TileContext(nc) as tc:
        with ExitStack() as ctx:
            konst = ctx.enter_context(tc.tile_pool(name="konst", bufs=1))
            cpool = ctx.enter_context(tc.tile_pool(name="cpool", bufs=2))
            qpool = ctx.enter_context(tc.tile_pool(name="qpool", bufs=2))
            ppool = ctx.enter_context(tc.tile_pool(name="ppool", bufs=2))
            ptpool = ctx.enter_context(tc.tile_pool(name="ptpool", bufs=2))
            xpool = ctx.enter_context(tc.tile_pool(name="xpool", bufs=2))
            rpool = ctx.enter_context(tc.tile_pool(name="rpool", bufs=2))
            opool = ctx.enter_context(tc.tile_pool(name="opool", bufs=2))
            small = ctx.enter_context(tc.tile_pool(name="small", bufs=4))
            s3_ps = ctx.enter_context(tc.tile_pool(name="s3_ps", bufs=2, space="PSUM"))
            tp_ps = ctx.enter_context(tc.tile_pool(name="tp_ps", bufs=1, space="PSUM"))
            x_ps = ctx.enter_context(tc.tile_pool(name="x_ps", bufs=1, space="PSUM"))
            mm_ps = ctx.enter_context(tc.tile_pool(name="mm_ps", bufs=4, space="PSUM"))

            kid_sb = konst.tile([128, 128], F16)
            nc.sync.dma_start(kid_sb[:], kid[:])
            nlam = konst.tile([128, 1], F32)
            nc.vector.memset(nlam[:], -LAM)

            def load_batch(b):
                csb = cpool.tile([128, KC * LC], F16, tag="csb")
                nc.sync.dma_start(
                    csb[:].rearrange("p (c i) -> p c i", c=KC), c16[b])
                q3sb = qpool.tile([128, KC * LQ], F16, tag="q3sb")
                nc.sync.dma_start(
                    q3sb[:].rearrange("p (c j) -> p c j", c=KC), q316[b])
                qtsb = qpool.tile([128, JC * H], F16, tag="qtsb")
                nc.sync.dma_start(
                    qtsb[:].rearrange("p (c h) -> p c h", c=JC), qt16[b])
                vqsb = qpool.tile([128, JC * 128], F16, tag="vqsb")
                nc.sync.dma_start(
                    vqsb[:].rearrange("p (c m) -> p c m", c=JC), vqec[b])
                ctsb = cpool.tile([128, IC * HA], F16, tag="ctsb")
                nc.sync.dma_start(
                    ctsb[:].rearrange("p (c h) -> p c h", c=IC), ct16[b])
                return csb, q3sb, qtsb, vqsb, ctsb

            tiles = load_batch(0)
            for b in range(NB):
                csb, q3sb, qtsb, vqsb, ctsb = tiles
                if b + 1 < NB:
                    tiles = load_batch(b + 1)
                c3 = csb[:].rearrange("p (c i) -> p c i", c=KC)
                q33 = q3sb[:].rearrange("p (c j) -> p c j", c=KC)
                qt3 = qtsb[:].rearrange("p (c h) -> p c h", c=JC)
                vq3 = vqsb[:].rearrange("p (c m) -> p c m", c=JC)
                ct3 = ctsb[:].rearrange("p (c h) -> p c h", c=IC)

                # ---- S3 [i,j] on PE -> exp -> P0 (fp16) ----
                p16 = ppool.tile([128, IC * LQ], F16, tag="p16")
                p3 = p16[:].rearrange("p (c j) -> p c j", c=IC)
                for g in range(IC // 2):
                    ps = s3_ps.tile([128, 512], F32, tag="s3")
                    for u in range(2):
                        ic = 2 * g + u
                        for kc in range(KC):
                            nc.tensor.matmul(
                                ps[:, u * LQ:(u + 1) * LQ],
                                c3[:, kc, ic * 128:(ic + 1) * 128],
                                q33[:, kc, :],
                                start=(kc == 0), stop=(kc == KC - 1))
                    nc.scalar.activation(
                        p16[:, g * 512:(g + 1) * 512], ps[:], AF.Exp,
                        bias=nlam[:])

                # ---- P0^T via PE transposes (fp16) ----
                p0t = ptpool.tile([128, JC * LC], F16, tag="p0t")
                pt3 = p0t[:].rearrange("p (c i) -> p c i", c=JC)
                for jc in range(JC):
                    for it in range(IT):
                        tp = tp_ps.tile([128, 512], F16, tag="tp")
                        for u in range(4):
                            ic = 4 * it + u
                            nc.tensor.transpose(
                                tp[:, u * 128:(u + 1) * 128],
                                p3[:, ic, jc * 128:(jc + 1) * 128],
                                kid_sb[:])
                        nc.vector.tensor_copy(
                            pt3[:, jc, it * 512:(it + 1) * 512], tp[:])

                # ---- replicated row sums -> reciprocal ----
                rrep = rpool.tile([128, LC], F32, tag="rrep")
                for it in range(IT):
                    rs = mm_ps.tile([128, 512], F32, tag="mm")
                    for jc in range(JC):
                        nc.tensor.matmul(
                            rs[:], vq3[:, jc, :],
                            pt3[:, jc, it * 512:(it + 1) * 512],
                            start=(jc == 0), stop=(jc == JC - 1))
                    nc.vector.reciprocal_approx_fast(
                        rrep[:, it * 512:(it + 1) * 512], rs[:])

                # ---- X_aug = P0^T(lhsT) @ [Ct*er | er]; normalize ----
                x16 = xpool.tile([128, JC * H], F16, tag="x16")
                x3 = x16[:].rearrange("p (c h) -> p c h", c=JC)
                for jc in range(JC):
                    xps = x_ps.tile([128, HA], F32, tag="x")
                    for ic in range(IC):
                        nc.tensor.matmul(
                            xps[:],
                            p3[:, ic, jc * 128:(jc + 1) * 128],
                            ct3[:, ic, :],
                            start=(ic == 0), stop=(ic == IC - 1))
                    colr = small.tile([128, 1], F32, tag="colr")
                    nc.vector.reciprocal_approx_fast(colr[:], xps[:, H:HA])
                    colr2 = small.tile([128, 1], F32, tag="colr2")
                    nc.vector.tensor_tensor(
                        colr2[:], colr[:], vq3[:, jc, 0:1], MUL)
                    nc.vector.tensor_scalar_mul(
                        x3[:, jc, :], xps[:, 0:H], colr2[:])

                # ---- A^T / Bt^T + normalized fp16 eviction ----
                asb = opool.tile([128, KC * LC], F16, tag="asb")
                bsb = opool.tile([128, KC * LC], F16, tag="bsb")
                a3 = asb[:].rearrange("p (c i) -> p c i", c=KC)
                b3 = bsb[:].rearrange("p (c i) -> p c i", c=KC)
                for hc in range(KC):
                    for it in range(IT):
                        i0, i1 = it * 512, (it + 1) * 512
                        aps = mm_ps.tile([128, 512], F32, tag="mm")
                        for jc in range(JC):
                            nc.tensor.matmul(
                                aps[:],
                                qt3[:, jc, hc * 128:(hc + 1) * 128],
                                pt3[:, jc, i0:i1],
                                start=(jc == 0), stop=(jc == JC - 1))
                        bps = mm_ps.tile([128, 512], F32, tag="mm")
                        for jc in range(JC):
                            nc.tensor.matmul(
                                bps[:],
                                x3[:, jc, hc * 128:(hc + 1) * 128],
                                pt3[:, jc, i0:i1],
                                start=(jc == 0), stop=(jc == JC - 1))
                        nc.vector.tensor_tensor(
                            a3[:, hc, i0:i1], aps[:], rrep[:, i0:i1], MUL)
                        nc.vector.tensor_tensor(
                            b3[:, hc, i0:i1], bps[:], rrep[:, i0:i1], MUL)

                nc.sync.dma_start(
                    a16[b], a3[:, :, :])
                nc.sync.dma_start(
                    b16[b], b3[:, :, :])

    nc.compile()
    return nc


def _prep(C, Q, cmask, qmask, line_project):
    w1, w2, w3 = np.split(line_project.astype(np.float64), 3)
    r = np.einsum('bhi,h->bi', C.astype(np.float64), w1).astype(np.float32)
    c_ = np.einsum('bhj,h->bj', Q.astype(np.float64), w2).astype(np.float32)
    vq = 1.0 - qmask
    vc = 1.0 - cmask
    ec = (np.exp(c_) * vq / S_EC).astype(np.float32)          # [B, LQ]
    er = (np.exp(r) * vc / S_ER).astype(np.float32)           # [B, LC]

    # [B, 128, KC, LC]: h = kc*128 + p
    c16 = np.ascontiguousarray(
        C.reshape(B, KC, 128, LC).transpose(0, 2, 1, 3)).astype(np.float16)
    q3 = (Q * w3.astype(np.float32)[None, :, None])
    q316 = np.ascontiguousarray(
        q3.reshape(B, KC, 128, LQ).transpose(0, 2, 1, 3)).astype(np.float16)
    # [B, 128, JC, H]: j = jc*128 + p
    qte = Q.transpose(0, 2, 1) * ec[:, :, None]               # [B, LQ, H]
    qt16 = np.ascontiguousarray(
        qte.reshape(B, JC, 128, H).transpose(0, 2, 1, 3)).astype(np.float16)
    vqec = np.ascontiguousarray(np.broadcast_to(
        ec.reshape(B, JC, 128).transpose(0, 2, 1)[:, :, :, None],
        (B, 128, JC, 128))).astype(np.float16)
    # [B, 128, IC, HA]: i = ic*128 + p
    cta = np.empty((B, LC, HA), np.float32)
    cta[:, :, 0:H] = C.transpose(0, 2, 1) * er[:, :, None]
    cta[:, :, H] = er
    ct16 = np.ascontiguousarray(
        cta.reshape(B, IC, 128, HA).transpose(0, 2, 1, 3)).astype(np.float16)
    return c16, q316, qt16, vqec, ct16


def make_in_maps(C, Q, cmask, qmask, line_project):
    C = np.asarray(C, dtype=np.float32)
    Q = np.asarray(Q, dtype=np.float32)
    cmask = np.asarray(cmask, dtype=np.float32)
    qmask = np.asarray(qmask, dtype=np.float32)
    line_project = np.asarray(line_project, dtype=np.float32)
    c16, q316, qt16, vqec, ct16 = _prep(C, Q, cmask, qmask, line_project)
    kid = np.eye(128, dtype=np.float16)
    in_maps = []
    for core in range(NCORES):
        s = slice(core * NB, (core + 1) * NB)
        in_maps.append({
            "c16": c16[s], "q316": q316[s], "qt16": qt16[s],
            "vqec": vqec[s], "ct16": ct16[s], "kid": kid,
        })
    return in_maps


def kernel(C, Q, cmask, qmask, line_project):
    from concourse.bass_utils import run_bass_kernel_spmd

    C = np.asarray(C, dtype=np.float32)
    in_maps = make_in_maps(C, Q, cmask, qmask, line_project)
    if "nc" not in _CACHE:
        _CACHE["nc"] = _build()
    nc = _CACHE["nc"]
    res = run_bass_kernel_spmd(nc, in_maps, core_ids=list(range(NCORES)))
    a16 = np.concatenate([res.results[c]["a16"] for c in range(NCORES)], axis=0)
    b16 = np.concatenate([res.results[c]["b16"] for c in range(NCORES)], axis=0)
    # [B, 128, KC, LC] (h = kc*128+p) -> [B, H, LC]
    A = a16.transpose(0, 2, 1, 3).reshape(B, H, LC).astype(np.float32)
    Bt = b16.transpose(0, 2, 1, 3).reshape(B, H, LC).astype(np.float32)
    out = np.empty((B, 4 * H, LC), np.float32)
    out[:, 0:H] = C
    out[:, H:2 * H] = A
    out[:, 2 * H:3 * H] = C * A
    out[:, 3 * H:4 * H] = C * Bt
    return out
